# revision 10
# baseline (speedup 1.0000x reference)
"""Trainium2 Bass kernel for EpisodicMemory farthest-kNN reward.

reference semantics:
    sq[b,m]  = max(|q_b - mem_m|^2, 0)
    mean     = mean(sq)                      (stop-grad running mean)
    kdist    = EPS / (sq/mean + EPS)         (monotone DECREASING in sq)
    top-k SMALLEST kdist == top-k LARGEST sq
    out[b]   = 1/sqrt(sum_k kdist + C)

Device work (the only O(B*M) part): dist'[b,m] = m2[m] - 2 q_b.mem_m and
per-query top-8 per 4096-row tile (vector-engine max8).  Everything else
(mean via analytic identity, final top-k merge + kernel formula) is O(B) or
O(M) host work.

Sharding: memory rows split contiguously across 8 cores; queries replicated.
Per-core candidates are gathered on host (8KB/core) and reduced to the
global bottom-k -- the all-gather of the sharded-kNN pattern done host-side.
"""

import os
import numpy as np
import ml_dtypes

import concourse.bass as bass
import concourse.mybir as mybir
import concourse.tile as tile
from concourse import bacc
from concourse.bass_utils import run_bass_kernel_spmd

# ---- problem constants (hardcoded per harness contract) ----
B, D = 64, 32
M = 2_000_000
N_CORES = 8
EPS = 1e-5
DENOM_C = 1e-5

MC = M // N_CORES                 # 250_000 rows per core
ROWS_PER_PSUM = 2048              # rows covered by one [128, 1024] psum tile
TILES = (MC + ROWS_PER_PSUM - 1) // ROWS_PER_PSUM     # 123
TILES += -TILES % 4               # multiple of 4 psum tiles per DMA tile -> 124
MC_PAD = TILES * ROWS_PER_PSUM    # 253_952
NPAIRS = MC_PAD // 2              # 126_976 moving columns per core
PSUM_F = 1024                     # psum tile free size (pairs per tile)
MM_F = 512                        # matmul free dim (one psum bank)
DMA_F = 4 * PSUM_F                # moving columns per DMA tile

BF16 = mybir.dt.bfloat16
F32 = mybir.dt.float32

_CACHE = {}


def _build_bass():
    nc = bacc.Bacc(
        "TRN2",
        target_bir_lowering=False,
        debug=False,
        num_devices=N_CORES,
    )

    # rhs rows 0..63: pair-packed memory dims; rows 64,65: m2 of the pair
    rhs_d = nc.dram_tensor("rhs", [66, NPAIRS], BF16, kind="ExternalInput")
    # stationary rows 0..63: block-diag -2 q^T; rows 64,65: ones selecting m2
    qstat_d = nc.dram_tensor("qstat", [66, 128], BF16, kind="ExternalInput")
    out_d = nc.dram_tensor("out", [128, 8 * TILES], F32, kind="ExternalOutput")

    with tile.TileContext(nc) as tc:
        with (
            tc.tile_pool(name="consts", bufs=1) as consts,
            tc.tile_pool(name="rhs", bufs=3) as rhs_pool,
            tc.tile_pool(name="cand", bufs=1) as cand_pool,
            tc.tile_pool(name="sbcp", bufs=3) as sb_pool,
            tc.tile_pool(name="merge", bufs=3) as mrg_pool,
            tc.tile_pool(name="psum", bufs=4, space="PSUM") as psum_pool,
        ):
            qstat = consts.tile([66, 128], BF16)
            nc.sync.dma_start(qstat[:], qstat_d[:, :])

            candbuf = cand_pool.tile([128, 8 * TILES], F32)

            rhs_t = None
            for t in range(TILES):
                if t % 4 == 0:
                    rhs_t = rhs_pool.tile([66, DMA_F], BF16, tag="rhs")
                    nc.sync.dma_start(
                        rhs_t[:], rhs_d[:, t * PSUM_F : t * PSUM_F + DMA_F]
                    )
                off = (t % 4) * PSUM_F

                psum_t = psum_pool.tile([128, PSUM_F], F32)
                for s in range(PSUM_F // MM_F):
                    nc.tensor.matmul(
                        psum_t[:, bass.ts(s, MM_F)],
                        qstat[:, :],
                        rhs_t[:, off + s * MM_F : off + (s + 1) * MM_F],
                        start=True,
                        stop=True,
                    )

                # top-8 of this tile per query-half.  The PSUM drain is
                # split across engines: DVE max8 direct on 1/3 of tiles;
                # the rest drain via ACT copy to SBUF bf16, then a DVE
                # pairwise max cascade in 2x bf16 mode + a small max8.
                if t % 3 == 0:
                    nc.vector.max(candbuf[:, bass.ts(t, 8)], psum_t[:, :])
                else:
                    sb_t = sb_pool.tile([128, PSUM_F], BF16)
                    nc.scalar.copy(sb_t[:], psum_t[:])
                    m1 = mrg_pool.tile([128, PSUM_F // 2], BF16, tag="m1")
                    nc.vector.tensor_max(
                        m1[:], sb_t[:, 0 : PSUM_F // 2], sb_t[:, PSUM_F // 2 :]
                    )
                    m2t = mrg_pool.tile([128, PSUM_F // 4], BF16, tag="m2")
                    nc.vector.tensor_max(
                        m2t[:], m1[:, 0 : PSUM_F // 4], m1[:, PSUM_F // 4 :]
                    )
                    nc.vector.max(candbuf[:, bass.ts(t, 8)], m2t[:, :])

            nc.sync.dma_start(out_d[:, :], candbuf[:])

    nc.compile()
    return nc


def _prep_inputs(query, memory):
    """Host-side shard + pack. Returns (in_maps, q2, mean_analytic)."""
    q = np.asarray(query, np.float32)
    mem = np.asarray(memory, np.float32)

    q2 = (q.astype(np.float64) ** 2).sum(1)            # [B]
    m2 = (mem.astype(np.float64) ** 2).sum(1)          # [M]
    mean_analytic = (
        q2.mean()
        + m2.mean()
        - 2.0 * np.dot(q.astype(np.float64).mean(0), mem.astype(np.float64).mean(0))
    )

    qstat = np.zeros((66, 128), np.float32)
    qstat[0:32, 0:64] = -2.0 * q.T
    qstat[32:64, 64:128] = -2.0 * q.T
    qstat[64, 0:64] = 1.0     # m2 of even row -> queries' even-half outputs
    qstat[65, 64:128] = 1.0   # m2 of odd row -> odd-half outputs
    qstat = qstat.astype(ml_dtypes.bfloat16)

    in_maps = []
    for c in range(N_CORES):
        rows = np.zeros((MC_PAD, D), np.float32)
        rows[:MC] = mem[c * MC : (c + 1) * MC]
        m2c = np.zeros(MC_PAD, np.float32)
        m2c[:MC] = m2[c * MC : (c + 1) * MC].astype(np.float32)

        rhs = np.empty((66, NPAIRS), np.float32)
        # rhs[p<64, n] = rows[2n + p//32, p%32]
        rhs[:64] = rows.reshape(NPAIRS, 2, D).transpose(1, 2, 0).reshape(64, NPAIRS)
        # rhs[64+r, n] = m2[2n + r]
        rhs[64:66] = m2c.reshape(NPAIRS, 2).T

        in_maps.append(
            {
                "rhs": np.ascontiguousarray(rhs.astype(ml_dtypes.bfloat16)),
                "qstat": qstat,
            }
        )
    return in_maps, q2, mean_analytic


def kernel(query, memory, k):
    k = int(k)
    assert k <= 12, f"per-tile top-8 candidate scheme validated for k<=12, got {k}"

    in_maps, q2, mean_analytic = _prep_inputs(query, memory)

    if "nc" not in _CACHE:
        _CACHE["nc"] = _build_bass()
    nc = _CACHE["nc"]

    trace = bool(int(os.environ.get("EPI_TRACE", "0")))
    res = run_bass_kernel_spmd(
        nc,
        in_maps,
        core_ids=list(range(N_CORES)),
        trace=trace,
    )
    _CACHE["last_result"] = res

    # host merge: gather per-core candidates, global bottom-k of kdist
    cands = np.concatenate(
        [
            np.concatenate([r["out"][0:64, :], r["out"][64:128, :]], axis=1)
            for r in res.results
        ],
        axis=1,
    )  # [64, 2*8*8*TILES]  (dist' = m2 - 2 q.m per candidate)

    idx = np.argpartition(cands, cands.shape[1] - k, axis=1)[:, -k:]
    sel = np.take_along_axis(cands, idx, axis=1).astype(np.float64)
    sq_sel = np.maximum(sel + q2[:, None], 0.0)
    kdist = EPS / (sq_sel / mean_analytic + EPS)
    reward = 1.0 / np.sqrt(kdist.sum(1) + DENOM_C)
    return reward.astype(np.float32)


# revision 12
# speedup vs baseline: 1.0005x; 1.0005x over previous
"""Trainium2 Bass kernel for EpisodicMemory farthest-kNN reward.

reference semantics:
    sq[b,m]  = max(|q_b - mem_m|^2, 0)
    mean     = mean(sq)                      (stop-grad running mean)
    kdist    = EPS / (sq/mean + EPS)         (monotone DECREASING in sq)
    top-k SMALLEST kdist == top-k LARGEST sq
    out[b]   = 1/sqrt(sum_k kdist + C)

Device work (the only O(B*M) part): dist'[b,m] = m2[m] - 2 q_b.mem_m and
per-query top-8 per 4096-row tile (vector-engine max8).  Everything else
(mean via analytic identity, final top-k merge + kernel formula) is O(B) or
O(M) host work.

Sharding: memory rows split contiguously across 8 cores; queries replicated.
Per-core candidates are gathered on host (8KB/core) and reduced to the
global bottom-k -- the all-gather of the sharded-kNN pattern done host-side.
"""

import os
import numpy as np
import ml_dtypes

import concourse.bass as bass
import concourse.mybir as mybir
import concourse.tile as tile
from concourse import bacc
from concourse.bass_utils import run_bass_kernel_spmd

# ---- problem constants (hardcoded per harness contract) ----
B, D = 64, 32
M = 2_000_000
N_CORES = 8
EPS = 1e-5
DENOM_C = 1e-5

MC = M // N_CORES                 # 250_000 rows per core
ROWS_PER_PSUM = 2048              # rows covered by one [128, 1024] psum tile
TILES = (MC + ROWS_PER_PSUM - 1) // ROWS_PER_PSUM     # 123
TILES += -TILES % 4               # multiple of 4 psum tiles per DMA tile -> 124
MC_PAD = TILES * ROWS_PER_PSUM    # 253_952
NPAIRS = MC_PAD // 2              # 126_976 moving columns per core
PSUM_F = 1024                     # psum tile free size (pairs per tile)
MM_F = 512                        # matmul free dim (one psum bank)
DMA_F = 4 * PSUM_F                # moving columns per DMA tile

BF16 = mybir.dt.bfloat16
FP8 = mybir.dt.float8e4
F32 = mybir.dt.float32
NP_FP8 = ml_dtypes.float8_e4m3fn

_CACHE = {}


def _build_bass():
    nc = bacc.Bacc(
        "TRN2",
        target_bir_lowering=False,
        debug=False,
        num_devices=N_CORES,
    )

    # fp8 moving operand, C=68: per column two 34-row blocks, each
    # [32 memory dims, m2/8 (hi), m2 residual (lo)] for rows 2n / 2n+1
    rhs_d = nc.dram_tensor("rhs", [68, NPAIRS], FP8, kind="ExternalInput")
    # stationary: block-diag [-2 q^T; 8*ones; ones] per half
    qstat_d = nc.dram_tensor("qstat", [68, 128], FP8, kind="ExternalInput")
    out_d = nc.dram_tensor("out", [128, 8 * TILES], F32, kind="ExternalOutput")

    with tile.TileContext(nc) as tc:
        with (
            tc.tile_pool(name="consts", bufs=1) as consts,
            tc.tile_pool(name="rhs", bufs=3) as rhs_pool,
            tc.tile_pool(name="cand", bufs=1) as cand_pool,
            tc.tile_pool(name="sbcp", bufs=3) as sb_pool,
            tc.tile_pool(name="merge", bufs=3) as mrg_pool,
            tc.tile_pool(name="psum", bufs=4, space="PSUM") as psum_pool,
        ):
            qstat = consts.tile([68, 128], FP8)
            nc.sync.dma_start(qstat[:], qstat_d[:, :])

            candbuf = cand_pool.tile([128, 8 * TILES], F32)

            rhs_t = None
            for t in range(TILES):
                if t % 4 == 0:
                    rhs_t = rhs_pool.tile([68, DMA_F], FP8, tag="rhs")
                    nc.sync.dma_start(
                        rhs_t[:], rhs_d[:, t * PSUM_F : t * PSUM_F + DMA_F]
                    )
                off = (t % 4) * PSUM_F

                psum_t = psum_pool.tile([128, PSUM_F], F32)
                for s in range(PSUM_F // MM_F):
                    nc.tensor.matmul(
                        psum_t[:, bass.ts(s, MM_F)],
                        qstat[:, :],
                        rhs_t[:, off + s * MM_F : off + (s + 1) * MM_F],
                        start=True,
                        stop=True,
                    )

                # top-8 of this tile per query-half.  The PSUM drain is
                # split across engines: DVE max8 direct on 1/3 of tiles;
                # the rest drain via ACT copy to SBUF bf16, then a DVE
                # pairwise max cascade in 2x bf16 mode + a small max8.
                if t % 3 == 0:
                    nc.vector.max(candbuf[:, bass.ts(t, 8)], psum_t[:, :])
                else:
                    sb_t = sb_pool.tile([128, PSUM_F], BF16)
                    nc.scalar.copy(sb_t[:], psum_t[:])
                    m1 = mrg_pool.tile([128, PSUM_F // 2], BF16, tag="m1")
                    nc.vector.tensor_max(
                        m1[:], sb_t[:, 0 : PSUM_F // 2], sb_t[:, PSUM_F // 2 :]
                    )
                    m2t = mrg_pool.tile([128, PSUM_F // 4], BF16, tag="m2")
                    nc.vector.tensor_max(
                        m2t[:], m1[:, 0 : PSUM_F // 4], m1[:, PSUM_F // 4 :]
                    )
                    nc.vector.max(candbuf[:, bass.ts(t, 8)], m2t[:, :])

            nc.sync.dma_start(out_d[:, :], candbuf[:])

    nc.compile()
    return nc


def _prep_inputs(query, memory):
    """Host-side shard + pack. Returns (in_maps, q2, mean_analytic)."""
    q = np.asarray(query, np.float32)
    mem = np.asarray(memory, np.float32)

    q2 = (q.astype(np.float64) ** 2).sum(1)            # [B]
    m2 = (mem.astype(np.float64) ** 2).sum(1)          # [M]
    mean_analytic = (
        q2.mean()
        + m2.mean()
        - 2.0 * np.dot(q.astype(np.float64).mean(0), mem.astype(np.float64).mean(0))
    )

    qstat = np.zeros((68, 128), np.float32)
    qstat[0:32, 0:64] = -2.0 * q.T      # even-row block
    qstat[32, 0:64] = 8.0               # m2_hi channel (scaled)
    qstat[33, 0:64] = 1.0               # m2_lo residual channel
    qstat[34:66, 64:128] = -2.0 * q.T   # odd-row block
    qstat[66, 64:128] = 8.0
    qstat[67, 64:128] = 1.0
    qstat = qstat.astype(NP_FP8)

    in_maps = []
    for c in range(N_CORES):
        rows = np.zeros((MC_PAD, D), np.float32)
        rows[:MC] = mem[c * MC : (c + 1) * MC]
        m2c = np.zeros(MC_PAD, np.float32)
        m2c[:MC] = m2[c * MC : (c + 1) * MC].astype(np.float32)

        m2hi = (m2c / 8.0).astype(NP_FP8).astype(np.float32)
        m2lo = (m2c - 8.0 * m2hi).astype(np.float32)

        rhs = np.empty((68, NPAIRS), np.float32)
        dims = rows.reshape(NPAIRS, 2, D)   # [n, r, d]
        rhs[0:32] = dims[:, 0, :].T         # even-row dims
        rhs[32] = m2hi.reshape(NPAIRS, 2)[:, 0]
        rhs[33] = m2lo.reshape(NPAIRS, 2)[:, 0]
        rhs[34:66] = dims[:, 1, :].T        # odd-row dims
        rhs[66] = m2hi.reshape(NPAIRS, 2)[:, 1]
        rhs[67] = m2lo.reshape(NPAIRS, 2)[:, 1]

        in_maps.append(
            {
                "rhs": np.ascontiguousarray(rhs.astype(NP_FP8)),
                "qstat": qstat,
            }
        )
    return in_maps, q2, mean_analytic


def kernel(query, memory, k):
    k = int(k)
    assert k <= 12, f"per-tile top-8 candidate scheme validated for k<=12, got {k}"

    in_maps, q2, mean_analytic = _prep_inputs(query, memory)

    if "nc" not in _CACHE:
        _CACHE["nc"] = _build_bass()
    nc = _CACHE["nc"]

    trace = bool(int(os.environ.get("EPI_TRACE", "0")))
    res = run_bass_kernel_spmd(
        nc,
        in_maps,
        core_ids=list(range(N_CORES)),
        trace=trace,
    )
    _CACHE["last_result"] = res

    # host merge: gather per-core candidates, global bottom-k of kdist
    cands = np.concatenate(
        [
            np.concatenate([r["out"][0:64, :], r["out"][64:128, :]], axis=1)
            for r in res.results
        ],
        axis=1,
    )  # [64, 2*8*8*TILES]  (dist' = m2 - 2 q.m per candidate)

    idx = np.argpartition(cands, cands.shape[1] - k, axis=1)[:, -k:]
    sel = np.take_along_axis(cands, idx, axis=1).astype(np.float64)
    sq_sel = np.maximum(sel + q2[:, None], 0.0)
    kdist = EPS / (sq_sel / mean_analytic + EPS)
    reward = 1.0 / np.sqrt(kdist.sum(1) + DENOM_C)
    return reward.astype(np.float32)


# revision 13
# speedup vs baseline: 1.0771x; 1.0766x over previous
"""Trainium2 Bass kernel for EpisodicMemory farthest-kNN reward.

reference semantics:
    sq[b,m]  = max(|q_b - mem_m|^2, 0)
    mean     = mean(sq)                      (stop-grad running mean)
    kdist    = EPS / (sq/mean + EPS)         (monotone DECREASING in sq)
    top-k SMALLEST kdist == top-k LARGEST sq
    out[b]   = 1/sqrt(sum_k kdist + C)

Device work (the only O(B*M) part): dist'[b,m] = m2[m] - 2 q_b.mem_m and
per-query top-8 per 4096-row tile (vector-engine max8).  Everything else
(mean via analytic identity, final top-k merge + kernel formula) is O(B) or
O(M) host work.

Sharding: memory rows split contiguously across 8 cores; queries replicated.
Per-core candidates are gathered on host (8KB/core) and reduced to the
global bottom-k -- the all-gather of the sharded-kNN pattern done host-side.
"""

import os
import numpy as np
import ml_dtypes

import concourse.bass as bass
import concourse.mybir as mybir
import concourse.tile as tile
from concourse import bacc
from concourse.bass_utils import run_bass_kernel_spmd

# ---- problem constants (hardcoded per harness contract) ----
B, D = 64, 32
M = 2_000_000
N_CORES = 8
EPS = 1e-5
DENOM_C = 1e-5

MC = M // N_CORES                 # 250_000 rows per core
ROWS_PER_PSUM = 2048              # rows covered by one [128, 1024] psum tile
TILES = (MC + ROWS_PER_PSUM - 1) // ROWS_PER_PSUM     # 123
TILES += -TILES % 4               # multiple of 4 psum tiles per DMA tile -> 124
MC_PAD = TILES * ROWS_PER_PSUM    # 253_952
NPAIRS = MC_PAD // 2              # 126_976 moving columns per core
PSUM_F = 1024                     # psum tile free size (pairs per tile)
MM_F = 512                        # matmul free dim (one psum bank)
DMA_F = 4 * PSUM_F                # moving columns per DMA tile

BF16 = mybir.dt.bfloat16
FP8 = mybir.dt.float8e4
F32 = mybir.dt.float32
NP_FP8 = ml_dtypes.float8_e4m3fn

_CACHE = {}


def _build_bass():
    nc = bacc.Bacc(
        "TRN2",
        target_bir_lowering=False,
        debug=False,
        num_devices=N_CORES,
    )

    # fp8 moving operand, C=68: per column two 34-row blocks, each
    # [32 memory dims, m2/8 (hi), m2 residual (lo)] for rows 2n / 2n+1
    rhs_d = nc.dram_tensor("rhs", [68, NPAIRS], FP8, kind="ExternalInput")
    # stationary: block-diag [-2 q^T; 8*ones; ones] per half
    qstat_d = nc.dram_tensor("qstat", [68, 128], FP8, kind="ExternalInput")
    out_d = nc.dram_tensor("out", [128, 8 * TILES], F32, kind="ExternalOutput")

    with tile.TileContext(nc) as tc:
        with (
            tc.tile_pool(name="consts", bufs=1) as consts,
            tc.tile_pool(name="rhs", bufs=3) as rhs_pool,
            tc.tile_pool(name="cand", bufs=1) as cand_pool,
            tc.tile_pool(name="sbcp", bufs=3) as sb_pool,
            tc.tile_pool(name="merge", bufs=3) as mrg_pool,
            tc.tile_pool(name="psum", bufs=4, space="PSUM") as psum_pool,
        ):
            qstat = consts.tile([68, 128], FP8)
            nc.sync.dma_start(qstat[:], qstat_d[:, :])

            candbuf = cand_pool.tile([128, 8 * TILES], F32)

            rhs_t = None
            for t in range(TILES):
                if t % 4 == 0:
                    rhs_t = rhs_pool.tile([68, DMA_F], FP8, tag="rhs")
                    nc.sync.dma_start(
                        rhs_t[:], rhs_d[:, t * PSUM_F : t * PSUM_F + DMA_F]
                    )
                off = (t % 4) * PSUM_F

                psum_t = psum_pool.tile([128, PSUM_F], F32)
                for s in range(PSUM_F // MM_F):
                    nc.tensor.matmul(
                        psum_t[:, bass.ts(s, MM_F)],
                        qstat[:, :],
                        rhs_t[:, off + s * MM_F : off + (s + 1) * MM_F],
                        start=True,
                        stop=True,
                    )

                # top-8 of this tile per query-half.  The PSUM drain is
                # split across engines: DVE max8 direct on 1/5 of tiles;
                # the rest drain via ACT copy to SBUF bf16, then a DVE
                # pairwise max cascade in 2x bf16 mode + a small max8.
                if t % 5 == 0:
                    nc.vector.max(candbuf[:, bass.ts(t, 8)], psum_t[:, :])
                else:
                    sb_t = sb_pool.tile([128, PSUM_F], BF16)
                    nc.scalar.copy(sb_t[:], psum_t[:])
                    m1 = mrg_pool.tile([128, PSUM_F // 2], BF16, tag="m1")
                    nc.vector.tensor_max(
                        m1[:], sb_t[:, 0 : PSUM_F // 2], sb_t[:, PSUM_F // 2 :]
                    )
                    m2t = mrg_pool.tile([128, PSUM_F // 4], BF16, tag="m2")
                    nc.vector.tensor_max(
                        m2t[:], m1[:, 0 : PSUM_F // 4], m1[:, PSUM_F // 4 :]
                    )
                    nc.vector.max(candbuf[:, bass.ts(t, 8)], m2t[:, :])

            nc.sync.dma_start(out_d[:, :], candbuf[:])

    nc.compile()
    return nc


def _prep_inputs(query, memory):
    """Host-side shard + pack. Returns (in_maps, q2, mean_analytic)."""
    q = np.asarray(query, np.float32)
    mem = np.asarray(memory, np.float32)

    q2 = (q.astype(np.float64) ** 2).sum(1)            # [B]
    m2 = (mem.astype(np.float64) ** 2).sum(1)          # [M]
    mean_analytic = (
        q2.mean()
        + m2.mean()
        - 2.0 * np.dot(q.astype(np.float64).mean(0), mem.astype(np.float64).mean(0))
    )

    qstat = np.zeros((68, 128), np.float32)
    qstat[0:32, 0:64] = -2.0 * q.T      # even-row block
    qstat[32, 0:64] = 8.0               # m2_hi channel (scaled)
    qstat[33, 0:64] = 1.0               # m2_lo residual channel
    qstat[34:66, 64:128] = -2.0 * q.T   # odd-row block
    qstat[66, 64:128] = 8.0
    qstat[67, 64:128] = 1.0
    qstat = qstat.astype(NP_FP8)

    in_maps = []
    for c in range(N_CORES):
        rows = np.zeros((MC_PAD, D), np.float32)
        rows[:MC] = mem[c * MC : (c + 1) * MC]
        m2c = np.zeros(MC_PAD, np.float32)
        m2c[:MC] = m2[c * MC : (c + 1) * MC].astype(np.float32)

        m2hi = (m2c / 8.0).astype(NP_FP8).astype(np.float32)
        m2lo = (m2c - 8.0 * m2hi).astype(np.float32)

        rhs = np.empty((68, NPAIRS), np.float32)
        dims = rows.reshape(NPAIRS, 2, D)   # [n, r, d]
        rhs[0:32] = dims[:, 0, :].T         # even-row dims
        rhs[32] = m2hi.reshape(NPAIRS, 2)[:, 0]
        rhs[33] = m2lo.reshape(NPAIRS, 2)[:, 0]
        rhs[34:66] = dims[:, 1, :].T        # odd-row dims
        rhs[66] = m2hi.reshape(NPAIRS, 2)[:, 1]
        rhs[67] = m2lo.reshape(NPAIRS, 2)[:, 1]

        in_maps.append(
            {
                "rhs": np.ascontiguousarray(rhs.astype(NP_FP8)),
                "qstat": qstat,
            }
        )
    return in_maps, q2, mean_analytic


def kernel(query, memory, k):
    k = int(k)
    assert k <= 12, f"per-tile top-8 candidate scheme validated for k<=12, got {k}"

    in_maps, q2, mean_analytic = _prep_inputs(query, memory)

    if "nc" not in _CACHE:
        _CACHE["nc"] = _build_bass()
    nc = _CACHE["nc"]

    trace = bool(int(os.environ.get("EPI_TRACE", "0")))
    res = run_bass_kernel_spmd(
        nc,
        in_maps,
        core_ids=list(range(N_CORES)),
        trace=trace,
    )
    _CACHE["last_result"] = res

    # host merge: gather per-core candidates, global bottom-k of kdist
    cands = np.concatenate(
        [
            np.concatenate([r["out"][0:64, :], r["out"][64:128, :]], axis=1)
            for r in res.results
        ],
        axis=1,
    )  # [64, 2*8*8*TILES]  (dist' = m2 - 2 q.m per candidate)

    idx = np.argpartition(cands, cands.shape[1] - k, axis=1)[:, -k:]
    sel = np.take_along_axis(cands, idx, axis=1).astype(np.float64)
    sq_sel = np.maximum(sel + q2[:, None], 0.0)
    kdist = EPS / (sq_sel / mean_analytic + EPS)
    reward = 1.0 / np.sqrt(kdist.sum(1) + DENOM_C)
    return reward.astype(np.float32)


# revision 14
# speedup vs baseline: 1.0863x; 1.0085x over previous
"""Trainium2 Bass kernel for EpisodicMemory farthest-kNN reward.

reference semantics:
    sq[b,m]  = max(|q_b - mem_m|^2, 0)
    mean     = mean(sq)                      (stop-grad running mean)
    kdist    = EPS / (sq/mean + EPS)         (monotone DECREASING in sq)
    top-k SMALLEST kdist == top-k LARGEST sq
    out[b]   = 1/sqrt(sum_k kdist + C)

Device work (the only O(B*M) part): dist'[b,m] = m2[m] - 2 q_b.mem_m and
per-query top-8 per 4096-row tile (vector-engine max8).  Everything else
(mean via analytic identity, final top-k merge + kernel formula) is O(B) or
O(M) host work.

Sharding: memory rows split contiguously across 8 cores; queries replicated.
Per-core candidates are gathered on host (8KB/core) and reduced to the
global bottom-k -- the all-gather of the sharded-kNN pattern done host-side.
"""

import os
import numpy as np
import ml_dtypes

import concourse.bass as bass
import concourse.mybir as mybir
import concourse.tile as tile
from concourse import bacc
from concourse.bass_utils import run_bass_kernel_spmd

# ---- problem constants (hardcoded per harness contract) ----
B, D = 64, 32
M = 2_000_000
N_CORES = 8
EPS = 1e-5
DENOM_C = 1e-5

MC = M // N_CORES                 # 250_000 rows per core
ROWS_PER_PSUM = 2048              # rows covered by one [128, 1024] psum tile
TILES = (MC + ROWS_PER_PSUM - 1) // ROWS_PER_PSUM     # 123
TILES += -TILES % 4               # multiple of 4 psum tiles per DMA tile -> 124
MC_PAD = TILES * ROWS_PER_PSUM    # 253_952
NPAIRS = MC_PAD // 2              # 126_976 moving columns per core
PSUM_F = 1024                     # psum tile free size (pairs per tile)
MM_F = 512                        # matmul free dim (one psum bank)
DMA_F = 4 * PSUM_F                # moving columns per DMA tile

BF16 = mybir.dt.bfloat16
FP8 = mybir.dt.float8e4
F32 = mybir.dt.float32
NP_FP8 = ml_dtypes.float8_e4m3fn

_CACHE = {}


def _build_bass():
    nc = bacc.Bacc(
        "TRN2",
        target_bir_lowering=False,
        debug=False,
        num_devices=N_CORES,
    )

    # fp8 moving operand, C=68: per column two 34-row blocks, each
    # [32 memory dims, m2/8 (hi), m2 residual (lo)] for rows 2n / 2n+1
    rhs_d = nc.dram_tensor("rhs", [68, NPAIRS], FP8, kind="ExternalInput")
    # stationary: block-diag [-2 q^T; 8*ones; ones] per half
    qstat_d = nc.dram_tensor("qstat", [68, 128], FP8, kind="ExternalInput")
    out_d = nc.dram_tensor("out", [128, 8 * TILES], F32, kind="ExternalOutput")

    with tile.TileContext(nc) as tc:
        with (
            tc.tile_pool(name="consts", bufs=1) as consts,
            tc.tile_pool(name="rhs", bufs=3) as rhs_pool,
            tc.tile_pool(name="cand", bufs=1) as cand_pool,
            tc.tile_pool(name="sbcp", bufs=3) as sb_pool,
            tc.tile_pool(name="merge", bufs=3) as mrg_pool,
            tc.tile_pool(name="psum", bufs=4, space="PSUM") as psum_pool,
        ):
            qstat = consts.tile([68, 128], FP8)
            nc.sync.dma_start(qstat[:], qstat_d[:, :])

            candbuf = cand_pool.tile([128, 8 * TILES], F32)

            rhs_t = None
            for t in range(TILES):
                if t % 4 == 0:
                    rhs_t = rhs_pool.tile([68, DMA_F], FP8, tag="rhs")
                    nc.sync.dma_start(
                        rhs_t[:], rhs_d[:, t * PSUM_F : t * PSUM_F + DMA_F]
                    )
                off = (t % 4) * PSUM_F

                psum_t = psum_pool.tile([128, PSUM_F], F32)
                for s in range(PSUM_F // MM_F):
                    nc.tensor.matmul(
                        psum_t[:, bass.ts(s, MM_F)],
                        qstat[:, :],
                        rhs_t[:, off + s * MM_F : off + (s + 1) * MM_F],
                        start=True,
                        stop=True,
                    )

                # top-8 of this tile per query-half.  The PSUM drain is
                # split across engines: DVE max8 direct on 1/5 of tiles;
                # the rest drain via ACT copy to SBUF bf16, then a DVE
                # pairwise max cascade in 2x bf16 mode + a small max8.
                if t % 6 == 0:
                    nc.vector.max(candbuf[:, bass.ts(t, 8)], psum_t[:, :])
                else:
                    sb_t = sb_pool.tile([128, PSUM_F], BF16)
                    nc.scalar.copy(sb_t[:], psum_t[:])
                    m1 = mrg_pool.tile([128, PSUM_F // 2], BF16, tag="m1")
                    nc.vector.tensor_max(
                        m1[:], sb_t[:, 0 : PSUM_F // 2], sb_t[:, PSUM_F // 2 :]
                    )
                    m2t = mrg_pool.tile([128, PSUM_F // 4], BF16, tag="m2")
                    nc.vector.tensor_max(
                        m2t[:], m1[:, 0 : PSUM_F // 4], m1[:, PSUM_F // 4 :]
                    )
                    nc.vector.max(candbuf[:, bass.ts(t, 8)], m2t[:, :])

            nc.sync.dma_start(out_d[:, :], candbuf[:])

    nc.compile()
    return nc


def _prep_inputs(query, memory):
    """Host-side shard + pack. Returns (in_maps, q2, mean_analytic)."""
    q = np.asarray(query, np.float32)
    mem = np.asarray(memory, np.float32)

    q2 = (q.astype(np.float64) ** 2).sum(1)            # [B]
    m2 = (mem.astype(np.float64) ** 2).sum(1)          # [M]
    mean_analytic = (
        q2.mean()
        + m2.mean()
        - 2.0 * np.dot(q.astype(np.float64).mean(0), mem.astype(np.float64).mean(0))
    )

    qstat = np.zeros((68, 128), np.float32)
    qstat[0:32, 0:64] = -2.0 * q.T      # even-row block
    qstat[32, 0:64] = 8.0               # m2_hi channel (scaled)
    qstat[33, 0:64] = 1.0               # m2_lo residual channel
    qstat[34:66, 64:128] = -2.0 * q.T   # odd-row block
    qstat[66, 64:128] = 8.0
    qstat[67, 64:128] = 1.0
    qstat = qstat.astype(NP_FP8)

    in_maps = []
    for c in range(N_CORES):
        rows = np.zeros((MC_PAD, D), np.float32)
        rows[:MC] = mem[c * MC : (c + 1) * MC]
        m2c = np.zeros(MC_PAD, np.float32)
        m2c[:MC] = m2[c * MC : (c + 1) * MC].astype(np.float32)

        m2hi = (m2c / 8.0).astype(NP_FP8).astype(np.float32)
        m2lo = (m2c - 8.0 * m2hi).astype(np.float32)

        rhs = np.empty((68, NPAIRS), np.float32)
        dims = rows.reshape(NPAIRS, 2, D)   # [n, r, d]
        rhs[0:32] = dims[:, 0, :].T         # even-row dims
        rhs[32] = m2hi.reshape(NPAIRS, 2)[:, 0]
        rhs[33] = m2lo.reshape(NPAIRS, 2)[:, 0]
        rhs[34:66] = dims[:, 1, :].T        # odd-row dims
        rhs[66] = m2hi.reshape(NPAIRS, 2)[:, 1]
        rhs[67] = m2lo.reshape(NPAIRS, 2)[:, 1]

        in_maps.append(
            {
                "rhs": np.ascontiguousarray(rhs.astype(NP_FP8)),
                "qstat": qstat,
            }
        )
    return in_maps, q2, mean_analytic


def kernel(query, memory, k):
    k = int(k)
    assert k <= 12, f"per-tile top-8 candidate scheme validated for k<=12, got {k}"

    in_maps, q2, mean_analytic = _prep_inputs(query, memory)

    if "nc" not in _CACHE:
        _CACHE["nc"] = _build_bass()
    nc = _CACHE["nc"]

    trace = bool(int(os.environ.get("EPI_TRACE", "0")))
    res = run_bass_kernel_spmd(
        nc,
        in_maps,
        core_ids=list(range(N_CORES)),
        trace=trace,
    )
    _CACHE["last_result"] = res

    # host merge: gather per-core candidates, global bottom-k of kdist
    cands = np.concatenate(
        [
            np.concatenate([r["out"][0:64, :], r["out"][64:128, :]], axis=1)
            for r in res.results
        ],
        axis=1,
    )  # [64, 2*8*8*TILES]  (dist' = m2 - 2 q.m per candidate)

    idx = np.argpartition(cands, cands.shape[1] - k, axis=1)[:, -k:]
    sel = np.take_along_axis(cands, idx, axis=1).astype(np.float64)
    sq_sel = np.maximum(sel + q2[:, None], 0.0)
    kdist = EPS / (sq_sel / mean_analytic + EPS)
    reward = 1.0 / np.sqrt(kdist.sum(1) + DENOM_C)
    return reward.astype(np.float32)
